# revision 10
# baseline (speedup 1.0000x reference)
"""Trainium2 Bass kernel for an LSTM language-model head.

Problem shapes (hardcoded): B=64, T=512, V=32000, E=256, H=512, n_cores=8.

Strategy
--------
Data-parallel over batch: each of the 8 NeuronCores handles BL=8 batch rows
end-to-end (no collectives).  Per core:

Phase 1 (memory/GEMM phase):
  - indirect-DMA gather of embedding rows for its 4096 tokens (token order
    n = t*BL + b), PE-transpose into x^T [E, n] (bf16),
  - one big GEMM xg^T = Wi_all^T.T @ x^T accumulated fp32 in PSUM, written
    to SBUF as bf16 in "gates-transposed" layout xgT[p, t*128 + m*8 + b]
    (m = 16 chunks of the 2048 gate dims, bias bi+bh folded in per-partition
    during the PSUM->SBUF copy).

Phase 2 (sequential recurrence, T steps):
  - gates^T[2048, BL] produced chunk-by-chunk with Wh^T chunks as the
    stationary operand (bf16 -> fast weight load) and h^T (bf16) moving,
  - all LSTM cell elementwise work runs in the transposed layout
    [128 partitions, 16*BL] so DVE/ACT use all 128 lanes,
  - cell state c stays fp32; h is kept bf16 (feeds next matmul).

Final: 4 small fp32 matmuls + sigmoid for the FC head; outputs DMAed in
device-natural layouts and re-assembled on host.
"""

import numpy as np
import ml_dtypes

B, T, V, E, H = 64, 512, 32000, 256, 512
NCORES = 8
BL = B // NCORES          # 8 local batch rows per core
NT = T * BL               # 4096 local tokens
MCH = 16                  # 2048 gate dims / 128
KCH = H // 128            # 4 contraction chunks for Wh
ECH = E // 128            # 2 contraction chunks for Wi
GW = MCH * BL             # 128: width of one step's gates^T tile
CW = KCH * BL             # 32: width of c/h state tiles


def build_program(t_steps=T, unroll=4, tail=8, nt=NT):
    """Build the SPMD Bass program (identical on every core)."""
    import concourse.bass as bass
    import concourse.bacc as bacc
    import concourse.mybir as mybir
    import concourse.tile as tile
    from concourse.bass import ds
    from concourse.masks import make_identity

    f32 = mybir.dt.float32
    bf16 = mybir.dt.bfloat16
    i32 = mybir.dt.int32
    AF = mybir.ActivationFunctionType
    OP = mybir.AluOpType

    nc = bacc.Bacc("TRN2", target_bir_lowering=False, debug=False,
                   enable_asserts=False, num_devices=NCORES)

    nidx = nt // 128
    # ---- DRAM I/O ----
    idx_d = nc.dram_tensor("idx", [128, nidx], i32, kind="ExternalInput")
    emb_d = nc.dram_tensor("emb", [V, E], f32, kind="ExternalInput")
    wiT_d = nc.dram_tensor("wiT", [ECH, MCH, 128, 128], bf16, kind="ExternalInput")
    bias_d = nc.dram_tensor("bias", [128, MCH], f32, kind="ExternalInput")
    whT_d = nc.dram_tensor("whT", [KCH, MCH, 128, 128], bf16, kind="ExternalInput")
    wfcT_d = nc.dram_tensor("wfcT", [128, KCH], f32, kind="ExternalInput")
    bfc_d = nc.dram_tensor("bfc", [1, 1], f32, kind="ExternalInput")
    hT_o = nc.dram_tensor("hT_out", [128, CW], f32, kind="ExternalOutput")
    o_o = nc.dram_tensor("o_out", [1, BL], f32, kind="ExternalOutput")

    with tile.TileContext(nc) as tc:
        with (
            tc.tile_pool(name="persist", bufs=1) as pp,
            tc.tile_pool(name="gather", bufs=4) as pg,
            tc.tile_pool(name="trps", bufs=2, space="PSUM") as ptr,
            tc.tile_pool(name="xgps", bufs=2, space="PSUM") as pxg,
            tc.tile_pool(name="gps", bufs=2, space="PSUM") as pgs,
            tc.tile_pool(name="fcps", bufs=1, space="PSUM") as pfcp,
            tc.tile_pool(name="work", bufs=2) as pw,
        ):
            # ---- persistent SBUF tensors ----
            idx_sb = pp.tile([128, nidx], i32)
            ident_gp = pp.tile([128, 128], bf16)
            ident = pp.tile([128, 128], bf16)
            wiT_sb = pp.tile([128, ECH * MCH * 128], bf16)
            bias_sb = pp.tile([128, MCH], f32)
            whT_sb = pp.tile([128, KCH * MCH * 128], bf16)
            wfcT_sb = pp.tile([128, KCH], f32)
            bfc_sb = pp.tile([1, 1], f32)
            xT_sb = pp.tile([128, ECH * nt], bf16)
            xgT_sb = pp.tile([128, t_steps * GW], bf16)
            c_sb = pp.tile([128, CW], f32)
            h_bf = pp.tile([128, CW], bf16)
            h_f32 = pp.tile([128, CW], f32)

            # ---- load weights / indices ----
            nc.sync.dma_start(out=idx_sb[:], in_=idx_d[:])
            for ec in range(ECH):
                for m in range(MCH):
                    nc.sync.dma_start(
                        out=wiT_sb[:, (ec * MCH + m) * 128:(ec * MCH + m + 1) * 128],
                        in_=wiT_d[ec, m],
                    )
            nc.sync.dma_start(out=bias_sb[:], in_=bias_d[:])
            for kc in range(KCH):
                for m in range(MCH):
                    nc.sync.dma_start(
                        out=whT_sb[:, (kc * MCH + m) * 128:(kc * MCH + m + 1) * 128],
                        in_=whT_d[kc, m],
                    )
            nc.sync.dma_start(out=wfcT_sb[:], in_=wfcT_d[:])
            nc.sync.dma_start(out=bfc_sb[:], in_=bfc_d[:])
            make_identity(nc, ident_gp[:])
            # Bounce the identity through DVE so every transpose's data deps
            # (identity, xbf cast, PSUM-slot WAR) live on a single proc:
            # walrus allows only one sync wait on a (self-loading) Matmult.
            nc.vector.tensor_copy(out=ident[:], in_=ident_gp[:])

            # ---- phase 1a: gather + transpose x ----
            for cch in range(nidx):
                xrow = pg.tile([128, E], f32, tag="xrow")
                xbf = pg.tile([128, E], bf16, tag="xbf")
                nc.gpsimd.indirect_dma_start(
                    out=xrow[:],
                    out_offset=None,
                    in_=emb_d[:],
                    in_offset=bass.IndirectOffsetOnAxis(
                        ap=idx_sb[:, cch:cch + 1], axis=0
                    ),
                )
                nc.vector.tensor_copy(out=xbf[:], in_=xrow[:])
                for ec in range(ECH):
                    pt = ptr.tile([128, 128], bf16, tag="tr")
                    nc.tensor.transpose(
                        out=pt[:], in_=xbf[:, ec * 128:(ec + 1) * 128],
                        identity=ident[:],
                    )
                    nc.vector.tensor_copy(
                        out=xT_sb[:, ec * nt + cch * 128: ec * nt + (cch + 1) * 128],
                        in_=pt[:],
                    )

            # ---- phase 1b: xg^T GEMM (+bias fold, bf16 store) ----
            tbs = min(512, nt)       # tokens per GEMM block
            tpb = tbs // BL          # t-steps per block
            ntb = nt // tbs
            for m in range(MCH):
                for tb in range(ntb):
                    pxt = pxg.tile([128, tbs], f32, tag="xg")
                    for ec in range(ECH):
                        nc.tensor.matmul(
                            out=pxt[:],
                            lhsT=wiT_sb[:, (ec * MCH + m) * 128:(ec * MCH + m + 1) * 128],
                            rhs=xT_sb[:, ec * nt + tb * tbs: ec * nt + (tb + 1) * tbs],
                            start=(ec == 0),
                            stop=(ec == ECH - 1),
                        )
                    # xgT[p, t*GW + m*BL + b] over t = tb*tpb .. tb*tpb+tpb-1
                    dst = xgT_sb[:].rearrange(
                        "p (t m b) -> p t m b", m=MCH, b=BL
                    )[:, tb * tpb:(tb + 1) * tpb, m, :]
                    nc.vector.tensor_scalar_add(
                        out=dst,
                        in0=pxt[:].rearrange("p (t b) -> p t b", b=BL),
                        scalar1=bias_sb[:, m:m + 1],
                    )

            # ---- phase 2: recurrence ----
            tc.strict_bb_all_engine_barrier()
            nc.tensor.nop()
            nc.vector.memset(c_sb[:], 0.0)
            nc.vector.memset(h_bf[:], 0.0)

            r_sl = slice(0, 4 * BL)            # chunks 0-3  (input gate r)
            f_sl = slice(4 * BL, 8 * BL)       # chunks 4-7  (forget gate)
            g_sl = slice(8 * BL, 12 * BL)      # chunks 8-11 (cell gate)
            o_sl = slice(12 * BL, 16 * BL)     # chunks 12-15 (output gate)

            def emit_step(toff, last=False):
                """toff: AP-offset expression (elements) of this step's xgT col."""
                ps = pgs.tile([128, GW], f32, tag="gps")
                for m in range(MCH):
                    for kc in range(KCH):
                        nc.tensor.matmul(
                            out=ps[:, m * BL:(m + 1) * BL],
                            lhsT=whT_sb[:, (kc * MCH + m) * 128:(kc * MCH + m + 1) * 128],
                            rhs=h_bf[:, kc * BL:(kc + 1) * BL],
                            start=(kc == 0),
                            stop=(kc == KCH - 1),
                        )
                gsb = pw.tile([128, GW], f32, tag="gsb")
                act = pw.tile([128, GW], f32, tag="act")
                half = 8 * BL
                nc.vector.tensor_tensor(
                    out=gsb[:, 0:half], in0=ps[:, 0:half],
                    in1=xgT_sb[:, ds(toff, half)], op=OP.add,
                )
                nc.vector.tensor_tensor(
                    out=gsb[:, half:GW], in0=ps[:, half:GW],
                    in1=xgT_sb[:, ds(toff + half, half)], op=OP.add,
                )
                nc.scalar.activation(act[:, r_sl.start:f_sl.stop],
                                     gsb[:, r_sl.start:f_sl.stop], AF.Sigmoid)
                nc.scalar.activation(act[:, g_sl], gsb[:, g_sl], AF.Tanh)
                nc.scalar.activation(act[:, o_sl], gsb[:, o_sl], AF.Sigmoid)
                t1 = pw.tile([128, CW], f32, tag="t1")
                t2 = pw.tile([128, CW], f32, tag="t2")
                tc_t = pw.tile([128, CW], f32, tag="tct")
                nc.vector.tensor_tensor(out=t1[:], in0=act[:, f_sl], in1=c_sb[:],
                                        op=OP.mult)
                nc.vector.tensor_tensor(out=t2[:], in0=act[:, r_sl], in1=act[:, g_sl],
                                        op=OP.mult)
                nc.vector.tensor_tensor(out=c_sb[:], in0=t1[:], in1=t2[:], op=OP.add)
                nc.scalar.activation(tc_t[:], c_sb[:], AF.Tanh)
                nc.vector.tensor_tensor(out=h_bf[:], in0=act[:, o_sl], in1=tc_t[:],
                                        op=OP.mult)
                if last:
                    nc.vector.tensor_tensor(out=h_f32[:], in0=act[:, o_sl],
                                            in1=tc_t[:], op=OP.mult)

            n_loop = t_steps - tail
            assert n_loop % unroll == 0
            if n_loop > 0:
                with tc.For_i(0, n_loop, unroll) as iv:
                    base = nc.vector.snap(iv * GW, min_val=0,
                                          max_val=(n_loop - unroll) * GW)
                    for u in range(unroll):
                        emit_step(base + u * GW)
            for t in range(n_loop, t_steps):
                emit_step(t * GW, last=(t == t_steps - 1))

            # ---- final FC head ----
            pfc = pfcp.tile([1, BL], f32, tag="fc")
            for kc in range(KCH):
                nc.tensor.matmul(
                    out=pfc[:],
                    lhsT=wfcT_sb[:, kc:kc + 1],
                    rhs=h_f32[:, kc * BL:(kc + 1) * BL],
                    start=(kc == 0),
                    stop=(kc == KCH - 1),
                )
            osb = pw.tile([1, BL], f32, tag="osb")
            nc.scalar.activation(osb[:], pfc[:], AF.Sigmoid, bias=bfc_sb[:, :1])
            nc.sync.dma_start(out=o_o[:], in_=osb[:])
            nc.sync.dma_start(out=hT_o[:], in_=h_f32[:])

    nc.compile()
    return nc


def prep_inputs(input_words, emb, Wi, bi, Wh, bh, Wfc, bfc, t_steps=T, nt=NT):
    """Host-side layout prep. Returns per-core in_maps."""
    bfl = ml_dtypes.bfloat16
    Wi_f = np.ascontiguousarray(Wi.reshape(4 * H, E), dtype=np.float32)
    Wh_f = np.ascontiguousarray(Wh.reshape(4 * H, H), dtype=np.float32)
    # wiT[ec, m, k, mm] = Wi_f[m*128+mm, ec*128+k]
    wiT = Wi_f.reshape(MCH, 128, ECH, 128).transpose(2, 0, 3, 1)
    wiT = np.ascontiguousarray(wiT, dtype=bfl)
    whT = Wh_f.reshape(MCH, 128, KCH, 128).transpose(2, 0, 3, 1)
    whT = np.ascontiguousarray(whT, dtype=bfl)
    # bias[mm, m] = (bi+bh)[m*128+mm]
    bias = np.ascontiguousarray(
        (bi + bh).reshape(4 * H).reshape(MCH, 128).T, dtype=np.float32)
    wfcT = np.ascontiguousarray(Wfc.reshape(KCH, 128).T, dtype=np.float32)
    bfc_a = np.ascontiguousarray(bfc.reshape(1, 1), dtype=np.float32)
    emb_f = np.ascontiguousarray(emb, dtype=np.float32)

    shared = dict(emb=emb_f, wiT=wiT, bias=bias, whT=whT, wfcT=wfcT, bfc=bfc_a)
    in_maps = []
    nidx = nt // 128
    for cid in range(NCORES):
        w = np.asarray(input_words[cid * BL:(cid + 1) * BL, :t_steps],
                       dtype=np.int64).astype(np.int32)
        tok = w.T.reshape(-1)                       # n = t*BL + b
        idx = np.ascontiguousarray(tok.reshape(nidx, 128).T, dtype=np.int32)
        in_maps.append(dict(idx=idx, **shared))
    return in_maps


def assemble_outputs(results):
    """results: list of per-core dicts with hT_out [128, CW], o_out [1, BL]."""
    hs, os_ = [], []
    for r in results:
        hT = np.asarray(r["hT_out"])
        h = hT.reshape(128, KCH, BL).transpose(2, 1, 0).reshape(BL, H)
        hs.append(h)
        os_.append(np.asarray(r["o_out"]).reshape(BL, 1))
    return np.concatenate(os_, 0).astype(np.float32), \
        np.concatenate(hs, 0).astype(np.float32)


_CACHE = {}


def kernel(input_words, emb, Wi, bi, Wh, bh, Wfc, bfc):
    from concourse.bass_utils import run_bass_kernel_spmd

    key = "prog"
    if key not in _CACHE:
        _CACHE[key] = build_program()
    nc = _CACHE[key]
    in_maps = prep_inputs(input_words, emb, Wi, bi, Wh, bh, Wfc, bfc)
    res = run_bass_kernel_spmd(nc, in_maps, core_ids=list(range(NCORES)))
    out, h = assemble_outputs(res.results)
    return out, h


# revision 13
# speedup vs baseline: 1.1596x; 1.1596x over previous
"""Trainium2 Bass kernel for an LSTM language-model head.

Problem shapes (hardcoded): B=64, T=512, V=32000, E=256, H=512, n_cores=8.

Strategy
--------
Data-parallel over batch: each of the 8 NeuronCores handles BL=8 batch rows
end-to-end (no collectives).  Per core:

Phase 1 (memory/GEMM phase):
  - indirect-DMA gather of embedding rows for its 4096 tokens (token order
    n = t*BL + b), PE-transpose into x^T [E, n] (bf16),
  - one big GEMM xg^T = Wi_all^T.T @ x^T accumulated fp32 in PSUM, written
    to SBUF as bf16 in "gates-transposed" layout xgT[p, t*128 + m*8 + b]
    (m = 16 chunks of the 2048 gate dims, bias bi+bh folded in per-partition
    during the PSUM->SBUF copy).

Phase 2 (sequential recurrence, T steps):
  - gates^T[2048, BL] produced chunk-by-chunk with Wh^T chunks as the
    stationary operand (bf16 -> fast weight load) and h^T (bf16) moving,
  - all LSTM cell elementwise work runs in the transposed layout
    [128 partitions, 16*BL] so DVE/ACT use all 128 lanes,
  - cell state c stays fp32; h is kept bf16 (feeds next matmul).

Final: 4 small fp32 matmuls + sigmoid for the FC head; outputs DMAed in
device-natural layouts and re-assembled on host.
"""

import numpy as np
import ml_dtypes

B, T, V, E, H = 64, 512, 32000, 256, 512
NCORES = 8
BL = B // NCORES          # 8 local batch rows per core
NT = T * BL               # 4096 local tokens
MCH = 16                  # 2048 gate dims / 128
KCH = H // 128            # 4 contraction chunks for Wh
ECH = E // 128            # 2 contraction chunks for Wi
GW = MCH * BL             # 128: width of one step's gates^T tile
CW = KCH * BL             # 32: width of c/h state tiles


def build_program(t_steps=T, unroll=4, tail=8, nt=NT):
    """Build the SPMD Bass program (identical on every core)."""
    import concourse.bass as bass
    import concourse.bacc as bacc
    import concourse.mybir as mybir
    import concourse.tile as tile
    from concourse.bass import ds
    from concourse.masks import make_identity

    f32 = mybir.dt.float32
    bf16 = mybir.dt.bfloat16
    i32 = mybir.dt.int32
    AF = mybir.ActivationFunctionType
    OP = mybir.AluOpType

    nc = bacc.Bacc("TRN2", target_bir_lowering=False, debug=False,
                   enable_asserts=False, num_devices=NCORES)

    nidx = nt // 128
    # ---- DRAM I/O ----
    idx_d = nc.dram_tensor("idx", [128, nidx], i32, kind="ExternalInput")
    emb_d = nc.dram_tensor("emb", [V, E], f32, kind="ExternalInput")
    wiT_d = nc.dram_tensor("wiT", [ECH, MCH, 128, 128], bf16, kind="ExternalInput")
    bias_d = nc.dram_tensor("bias", [128, MCH], f32, kind="ExternalInput")
    whT_d = nc.dram_tensor("whT", [KCH, MCH, 128, 128], bf16, kind="ExternalInput")
    wfcT_d = nc.dram_tensor("wfcT", [128, KCH], f32, kind="ExternalInput")
    bfc_d = nc.dram_tensor("bfc", [1, 1], f32, kind="ExternalInput")
    hT_o = nc.dram_tensor("hT_out", [128, CW], f32, kind="ExternalOutput")
    o_o = nc.dram_tensor("o_out", [1, BL], f32, kind="ExternalOutput")

    with tile.TileContext(nc) as tc:
        with (
            tc.tile_pool(name="persist", bufs=1) as pp,
            tc.tile_pool(name="gather", bufs=4) as pg,
            tc.tile_pool(name="trps", bufs=2, space="PSUM") as ptr,
            tc.tile_pool(name="xgps", bufs=2, space="PSUM") as pxg,
            tc.tile_pool(name="gps", bufs=1, space="PSUM") as pgs,
            tc.tile_pool(name="work", bufs=2) as pw,
        ):
            # ---- persistent SBUF tensors ----
            idx_sb = pp.tile([128, nidx], i32)
            ident_gp = pp.tile([128, 128], bf16)
            ident = pp.tile([128, 128], bf16)
            wiT_sb = pp.tile([128, ECH * MCH * 128], bf16)
            bias_sb = pp.tile([128, MCH], f32)
            whT_sb = pp.tile([128, KCH * MCH * 128], bf16)
            wfcT_sb = pp.tile([128, KCH], f32)
            bfc_sb = pp.tile([1, 1], f32)
            xT_sb = pp.tile([128, ECH * nt], bf16)
            xgT_sb = pp.tile([128, t_steps * GW], bf16)
            c_sb = pp.tile([128, CW], f32)
            h_bf = pp.tile([128, CW], bf16)
            h_f32 = pp.tile([128, CW], f32)

            # ---- load weights / indices ----
            nc.sync.dma_start(out=idx_sb[:], in_=idx_d[:])
            for ec in range(ECH):
                for m in range(MCH):
                    nc.sync.dma_start(
                        out=wiT_sb[:, (ec * MCH + m) * 128:(ec * MCH + m + 1) * 128],
                        in_=wiT_d[ec, m],
                    )
            nc.sync.dma_start(out=bias_sb[:], in_=bias_d[:])
            for kc in range(KCH):
                for m in range(MCH):
                    nc.sync.dma_start(
                        out=whT_sb[:, (kc * MCH + m) * 128:(kc * MCH + m + 1) * 128],
                        in_=whT_d[kc, m],
                    )
            nc.sync.dma_start(out=wfcT_sb[:], in_=wfcT_d[:])
            nc.sync.dma_start(out=bfc_sb[:], in_=bfc_d[:])
            make_identity(nc, ident_gp[:])
            # Bounce the identity through DVE so every transpose's data deps
            # (identity, xbf cast, PSUM-slot WAR) live on a single proc:
            # walrus allows only one sync wait on a (self-loading) Matmult.
            nc.vector.tensor_copy(out=ident[:], in_=ident_gp[:])

            # ---- phase 1a: gather + transpose x ----
            for cch in range(nidx):
                xrow = pg.tile([128, E], f32, tag="xrow")
                xbf = pg.tile([128, E], bf16, tag="xbf")
                nc.gpsimd.indirect_dma_start(
                    out=xrow[:],
                    out_offset=None,
                    in_=emb_d[:],
                    in_offset=bass.IndirectOffsetOnAxis(
                        ap=idx_sb[:, cch:cch + 1], axis=0
                    ),
                )
                nc.vector.tensor_copy(out=xbf[:], in_=xrow[:])
                for ec in range(ECH):
                    pt = ptr.tile([128, 128], bf16, tag="tr")
                    nc.tensor.transpose(
                        out=pt[:], in_=xbf[:, ec * 128:(ec + 1) * 128],
                        identity=ident[:],
                    )
                    nc.vector.tensor_copy(
                        out=xT_sb[:, ec * nt + cch * 128: ec * nt + (cch + 1) * 128],
                        in_=pt[:],
                    )

            # ---- phase 1b: xg^T GEMM (+bias fold, bf16 store) ----
            tbs = min(512, nt)       # tokens per GEMM block
            tpb = tbs // BL          # t-steps per block
            ntb = nt // tbs
            for m in range(MCH):
                for tb in range(ntb):
                    pxt = pxg.tile([128, tbs], f32, tag="xg")
                    for ec in range(ECH):
                        nc.tensor.matmul(
                            out=pxt[:],
                            lhsT=wiT_sb[:, (ec * MCH + m) * 128:(ec * MCH + m + 1) * 128],
                            rhs=xT_sb[:, ec * nt + tb * tbs: ec * nt + (tb + 1) * tbs],
                            start=(ec == 0),
                            stop=(ec == ECH - 1),
                        )
                    # xgT[p, t*GW + m*BL + b] over t = tb*tpb .. tb*tpb+tpb-1
                    dst = xgT_sb[:].rearrange(
                        "p (t m b) -> p t m b", m=MCH, b=BL
                    )[:, tb * tpb:(tb + 1) * tpb, m, :]
                    nc.vector.tensor_scalar_add(
                        out=dst,
                        in0=pxt[:].rearrange("p (t b) -> p t b", b=BL),
                        scalar1=bias_sb[:, m:m + 1],
                    )

            # ---- phase 2: recurrence ----
            tc.strict_bb_all_engine_barrier()
            nc.tensor.nop()
            nc.vector.memset(c_sb[:], 0.0)
            nc.vector.memset(h_bf[:], 0.0)

            # gate-chunk order in whT/bias/xgT is r, g, f, o (host perm) so
            # each gate's elementwise work overlaps the next gate's matmuls;
            # each gate gets its own PSUM tile (distinct bank) so DVE reads
            # don't serialize against PE writes of later gates.
            QW = 4 * BL  # 32 cols per gate

            def emit_step(toff, last=False):
                pss = [pgs.tile([128, QW], f32, tag=f"ps{q}", name=f"ps{q}")
                       for q in range(4)]
                gsb = pw.tile([128, GW], f32, tag="gsb")
                act = pw.tile([128, GW], f32, tag="act")
                t1 = pw.tile([128, CW], f32, tag="t1")
                t2 = pw.tile([128, CW], f32, tag="t2")
                tc_t = pw.tile([128, CW], f32, tag="tct")

                def mm_gate(q):
                    for mj in range(4):
                        m = q * 4 + mj
                        for kc in range(KCH):
                            nc.tensor.matmul(
                                out=pss[q][:, mj * BL:(mj + 1) * BL],
                                lhsT=whT_sb[:, (kc * MCH + m) * 128:(kc * MCH + m + 1) * 128],
                                rhs=h_bf[:, kc * BL:(kc + 1) * BL],
                                start=(kc == 0),
                                stop=(kc == KCH - 1),
                            )

                def add_xg(q):
                    nc.vector.tensor_tensor(
                        out=gsb[:, q * QW:(q + 1) * QW], in0=pss[q][:],
                        in1=xgT_sb[:, ds(toff + q * QW, QW)], op=OP.add,
                    )

                r_c, g_c, f_c, o_c = (slice(q * QW, (q + 1) * QW) for q in range(4))
                mm_gate(0)                 # r
                add_xg(0)
                nc.scalar.activation(act[:, r_c], gsb[:, r_c], AF.Sigmoid)
                mm_gate(1)                 # g (cell)
                add_xg(1)
                nc.scalar.activation(act[:, g_c], gsb[:, g_c], AF.Tanh)
                nc.vector.tensor_tensor(out=t2[:], in0=act[:, r_c],
                                        in1=act[:, g_c], op=OP.mult)
                mm_gate(2)                 # f
                add_xg(2)
                nc.scalar.activation(act[:, f_c], gsb[:, f_c], AF.Sigmoid)
                nc.vector.tensor_tensor(out=t1[:], in0=act[:, f_c], in1=c_sb[:],
                                        op=OP.mult)
                nc.vector.tensor_tensor(out=c_sb[:], in0=t1[:], in1=t2[:],
                                        op=OP.add)
                nc.scalar.activation(tc_t[:], c_sb[:], AF.Tanh)
                mm_gate(3)                 # o
                add_xg(3)
                nc.scalar.activation(act[:, o_c], gsb[:, o_c], AF.Sigmoid)
                nc.vector.tensor_tensor(out=h_bf[:], in0=act[:, o_c],
                                        in1=tc_t[:], op=OP.mult)
                if last:
                    nc.vector.tensor_tensor(out=h_f32[:], in0=act[:, o_c],
                                            in1=tc_t[:], op=OP.mult)

            n_loop = t_steps - tail
            assert n_loop % unroll == 0
            if n_loop > 0:
                with tc.For_i(0, n_loop, unroll) as iv:
                    base = nc.vector.snap(iv * GW, min_val=0,
                                          max_val=(n_loop - unroll) * GW)
                    for u in range(unroll):
                        emit_step(base + u * GW)
            for t in range(n_loop, t_steps):
                emit_step(t * GW, last=(t == t_steps - 1))

            # ---- final FC head ----
            pfc = pgs.tile([1, BL], f32, tag="ps0")
            for kc in range(KCH):
                nc.tensor.matmul(
                    out=pfc[:],
                    lhsT=wfcT_sb[:, kc:kc + 1],
                    rhs=h_f32[:, kc * BL:(kc + 1) * BL],
                    start=(kc == 0),
                    stop=(kc == KCH - 1),
                )
            osb = pw.tile([1, BL], f32, tag="osb")
            nc.scalar.activation(osb[:], pfc[:], AF.Sigmoid, bias=bfc_sb[:, :1])
            nc.sync.dma_start(out=o_o[:], in_=osb[:])
            nc.sync.dma_start(out=hT_o[:], in_=h_f32[:])

    nc.compile()
    return nc


def prep_inputs(input_words, emb, Wi, bi, Wh, bh, Wfc, bfc, t_steps=T, nt=NT):
    """Host-side layout prep. Returns per-core in_maps."""
    bfl = ml_dtypes.bfloat16
    perm = [0, 2, 1, 3]          # device gate order: r, g(cell), f, o
    Wi_f = np.ascontiguousarray(
        np.asarray(Wi, dtype=np.float32)[perm].reshape(4 * H, E))
    Wh_f = np.ascontiguousarray(
        np.asarray(Wh, dtype=np.float32)[perm].reshape(4 * H, H))
    # wiT[ec, m, k, mm] = Wi_f[m*128+mm, ec*128+k]
    wiT = Wi_f.reshape(MCH, 128, ECH, 128).transpose(2, 0, 3, 1)
    wiT = np.ascontiguousarray(wiT, dtype=bfl)
    whT = Wh_f.reshape(MCH, 128, KCH, 128).transpose(2, 0, 3, 1)
    whT = np.ascontiguousarray(whT, dtype=bfl)
    # bias[mm, m] = (bi+bh)[m*128+mm]
    bias = np.ascontiguousarray(
        np.asarray(bi + bh, dtype=np.float32)[perm].reshape(MCH, 128).T)
    wfcT = np.ascontiguousarray(Wfc.reshape(KCH, 128).T, dtype=np.float32)
    bfc_a = np.ascontiguousarray(bfc.reshape(1, 1), dtype=np.float32)
    emb_f = np.ascontiguousarray(emb, dtype=np.float32)

    shared = dict(emb=emb_f, wiT=wiT, bias=bias, whT=whT, wfcT=wfcT, bfc=bfc_a)
    in_maps = []
    nidx = nt // 128
    for cid in range(NCORES):
        w = np.asarray(input_words[cid * BL:(cid + 1) * BL, :t_steps],
                       dtype=np.int64).astype(np.int32)
        tok = w.T.reshape(-1)                       # n = t*BL + b
        idx = np.ascontiguousarray(tok.reshape(nidx, 128).T, dtype=np.int32)
        in_maps.append(dict(idx=idx, **shared))
    return in_maps


def assemble_outputs(results):
    """results: list of per-core dicts with hT_out [128, CW], o_out [1, BL]."""
    hs, os_ = [], []
    for r in results:
        hT = np.asarray(r["hT_out"])
        h = hT.reshape(128, KCH, BL).transpose(2, 1, 0).reshape(BL, H)
        hs.append(h)
        os_.append(np.asarray(r["o_out"]).reshape(BL, 1))
    return np.concatenate(os_, 0).astype(np.float32), \
        np.concatenate(hs, 0).astype(np.float32)


_CACHE = {}


def kernel(input_words, emb, Wi, bi, Wh, bh, Wfc, bfc):
    from concourse.bass_utils import run_bass_kernel_spmd

    key = "prog"
    if key not in _CACHE:
        _CACHE[key] = build_program()
    nc = _CACHE[key]
    in_maps = prep_inputs(input_words, emb, Wi, bi, Wh, bh, Wfc, bfc)
    res = run_bass_kernel_spmd(nc, in_maps, core_ids=list(range(NCORES)))
    out, h = assemble_outputs(res.results)
    return out, h
